# revision 1
# baseline (speedup 1.0000x reference)
"""ConvexUpsampler Trainium2 kernel.

Computes, per batch element b (one NeuronCore each, 8 cores):
  x    = relu(conv3x3(feat, w1) + b1)            # [256, 96, 96]
  m    = conv1x1(x, w2) + b2                     # [576, 96, 96]
  mask = softmax over k of m.reshape(9, 64, ...) # k = 3x3 tap index
  up   = sum_k mask[k,pq,hw] * unfold(flow)[c,k,hw] * 8
  out  = pixel-shuffle(up)                       # [2, 768, 768]

Strategy: data-parallel over batch (8 cores).  Convs run on the PE as
accumulated matmuls (fp32r operands).  conv2 is "swapped" so its PSUM
output has pixels on partitions, which lets the softmax-weighted convex
combination run as wide DVE ops with the unfolded flow entering via
free-dim broadcast APs.  Flow unfolding + all weight re-layouts are done
host-side in numpy (cheap, tiny tensors).
"""

import os
import sys
from contextlib import ExitStack

# The kernel needs the axon-tunneled trn2 devices; a CPU pin (as some jax
# reference harnesses set) would hide them.  Must happen before jax loads.
if os.environ.get("JAX_PLATFORMS", "").strip().lower() == "cpu":
    os.environ["JAX_PLATFORMS"] = ""

sys.path.insert(0, "/opt/trn_rl_repo")

import numpy as np

import concourse.bass as bass  # noqa: E402
import concourse.tile as tile  # noqa: E402
from concourse import bacc, mybir  # noqa: E402
from concourse.bass_utils import run_bass_kernel_spmd  # noqa: E402

F32 = mybir.dt.float32
F32R = mybir.dt.float32r

B = 8
C = 256
H = W = 96
UP = 8
PIX = H * W          # 9216
GW = 98              # padded grid width
NBAND = 24           # bands of 4 output rows
BAND_N = 4 * GW      # matmul free size for conv1 (392)
NCHUNK = PIX // 128  # 72 pixel chunks of 128

# matmul operand dtype knob: "f32r" (full speed), "bf16" (full speed, lower
# precision), or "f32" (4x slower, exact).  The whole producer chain (DRAM
# decl -> DMA -> SBUF tile -> ACT output) is declared in this dtype: the
# walrus verifier requires fp32r matmul operands to be *produced* as fp32r.
BF16 = mybir.dt.bfloat16
MM_NAME = os.environ.get("CONVUP_MM_DT", "f32r")
MM_DT = {"f32r": F32R, "f32": F32, "bf16": BF16}[MM_NAME]


def _mm_np(a):
    """Host-side array in the dtype matching the MM_DT DRAM declarations."""
    if MM_NAME == "bf16":
        import ml_dtypes

        return np.ascontiguousarray(a).astype(ml_dtypes.bfloat16)
    return np.ascontiguousarray(a, np.float32)


def _chunk_segments(j):
    """Split pixel chunk j (pixels 128j..128j+127, h-major) into runs with a
    single output row each: (i0, n, h, w0)."""
    segs = []
    i = 0
    while i < 128:
        pix = 128 * j + i
        h, w0 = divmod(pix, W)
        n = min(128 - i, W - w0)
        segs.append((i, n, h, w0))
        i += n
    return segs


def _build(with_b2: bool, reps: int = 1, with_b1: bool = True):
    nc = bacc.Bacc("TRN2", target_bir_lowering=False, debug=False)
    featp = nc.dram_tensor("featp", [128, 2, 100 * GW], MM_DT, kind="ExternalInput").ap()
    w1s = nc.dram_tensor("w1s", [128, 2, 9, 2, 128], MM_DT, kind="ExternalInput").ap()
    w2s = nc.dram_tensor("w2s", [128, 2, 576], MM_DT, kind="ExternalInput").ap()
    b1s = nc.dram_tensor("b1s", [128, 2], F32, kind="ExternalInput").ap()
    fdat = nc.dram_tensor("fdat", [128, NCHUNK, 18], F32, kind="ExternalInput").ap()
    b2s = None
    if with_b2:
        b2s = nc.dram_tensor("b2s", [1, 576], MM_DT, kind="ExternalInput").ap()
    out = nc.dram_tensor("out", [2, 768, 768], F32, kind="ExternalOutput").ap()
    # out viewed as [c, hh, p, ww, q] for the pixel-shuffle scatter store
    out_v = out.rearrange("c (hh p) (ww q) -> c hh p ww q", p=UP, q=UP)

    with tile.TileContext(nc) as tc, ExitStack() as ctx:
        cpool = ctx.enter_context(tc.tile_pool(name="const", bufs=1))
        xpool = ctx.enter_context(tc.tile_pool(name="x", bufs=3))
        epool = ctx.enter_context(tc.tile_pool(name="e", bufs=3))
        ppool = ctx.enter_context(tc.tile_pool(name="prod", bufs=2))
        npool = ctx.enter_context(tc.tile_pool(name="num", bufs=2))
        dpool = ctx.enter_context(tc.tile_pool(name="d", bufs=2))
        upool = ctx.enter_context(tc.tile_pool(name="up", bufs=3))
        psum1 = ctx.enter_context(tc.tile_pool(name="ps1", bufs=2, space="PSUM"))
        psum2 = ctx.enter_context(tc.tile_pool(name="ps2", bufs=2, space="PSUM"))

        feat_sb = cpool.tile([128, 2, 100 * GW], MM_DT, tag="feat")
        # split the big feat load so early conv1 bands start sooner
        for kc in range(2):
            for s0 in range(0, 100 * GW, 25 * GW):
                nc.sync.dma_start(
                    feat_sb[:, kc, s0 : s0 + 25 * GW],
                    featp[:, kc, s0 : s0 + 25 * GW],
                )
        w1_sb = cpool.tile([128, 2, 9, 2, 128], MM_DT, tag="w1")
        nc.sync.dma_start(w1_sb[:], w1s[:])
        w2_sb = cpool.tile([128, 2, 576], MM_DT, tag="w2")
        nc.sync.dma_start(w2_sb[:], w2s[:])
        b1_sb = cpool.tile([128, 2], F32, tag="b1")
        nc.sync.dma_start(b1_sb[:], b1s[:])
        f_sb = cpool.tile([128, NCHUNK, 18], F32, tag="fdat")
        nc.sync.dma_start(f_sb[:], fdat[:])
        if with_b2:
            b2_sb = cpool.tile([1, 576], MM_DT, tag="b2")
            nc.sync.dma_start(b2_sb[:], b2s[:])
            ones_sb = cpool.tile([1, 128], MM_DT, tag="ones")
            nc.vector.memset(ones_sb[:], 1.0)

        def conv1_band(hb):
            r0 = 4 * hb + 1  # first output grid row of this band
            # --- conv1: 3x3x256->256 over 4 rows (padded width) ---
            # both mc halves accumulate into one 2-bank psum tile
            ps = psum1.tile([128, 2, 512], F32, tag="ps1")
            for mc in range(2):
                first = True
                for kc in range(2):
                    for tap in range(9):
                        dh, dw = divmod(tap, 3)
                        s = (r0 + dh) * GW + dw - 1
                        nc.tensor.matmul(
                            ps[:, mc, :BAND_N],
                            lhsT=w1_sb[:, kc, tap, mc, :],
                            rhs=feat_sb[:, kc, s : s + BAND_N],
                            start=first,
                            stop=(kc == 1 and tap == 8),
                        )
                        first = False
            # relu(x + b1), compacting 98-wide padded rows to 96
            xt = xpool.tile([128, 2, 4 * W], MM_DT, tag="x")
            psv = ps[:, :, :BAND_N].rearrange("p m (r c) -> p m r c", c=GW)[
                :, :, :, 1 : W + 1
            ]
            if with_b1:
                for mc in range(2):
                    nc.scalar.activation(
                        out=xt[:, mc].rearrange("p (r c) -> p r c", c=W),
                        in_=psv[:, mc],
                        func=mybir.ActivationFunctionType.Relu,
                        bias=b1_sb[:, mc : mc + 1],
                    )
            else:
                nc.scalar.activation(
                    out=xt[:].rearrange("p m (r c) -> p m r c", c=W),
                    in_=psv,
                    func=mybir.ActivationFunctionType.Relu,
                )
            return xt

        def tail_band(hb, xb):
            # --- conv2 (swapped: pixels on partitions) + softmax + convex ---
            e_b = epool.tile([128, 3, 576], F32, tag="e")
            for t in range(3):
                ps2 = psum2.tile([128, 2, 512], F32, tag="ps2")
                for half in range(2):
                    for kc in range(2):
                        nc.tensor.matmul(
                            ps2[:, half, :288],
                            lhsT=xb[:, kc, t * 128 : (t + 1) * 128],
                            rhs=w2_sb[:, kc, half * 288 : (half + 1) * 288],
                            start=(kc == 0),
                            stop=(kc == 1 and not with_b2),
                        )
                    if with_b2:
                        nc.tensor.matmul(
                            ps2[:, half, :288],
                            lhsT=ones_sb[:, :],
                            rhs=b2_sb[:, half * 288 : (half + 1) * 288],
                            start=False,
                            stop=True,
                        )
                nc.scalar.activation(
                    out=e_b[:, t, :].rearrange("p (h n) -> p h n", h=2),
                    in_=ps2[:, :, :288],
                    func=mybir.ActivationFunctionType.Exp,
                )
            # band-wide views [128, 3, 64, 9]
            e4 = e_b[:].rearrange("p t (q k) -> p t q k", k=9)

            num_b = npool.tile([128, 3, 2, 64], F32, tag="num")
            for c in range(2):
                pr = ppool.tile([128, 3, 576], F32, tag=f"prod{c}")
                prv = pr[:].rearrange("p t (q k) -> p t q k", k=9)
                fb = f_sb[:, 3 * hb : 3 * hb + 3, None, c * 9 : c * 9 + 9]
                fb = fb.to_broadcast((128, 3, 64, 9))
                # products on GPSIMD (frees the vector engine)
                mul_eng = (
                    nc.vector if os.environ.get("CONVUP_NO_GPS") else nc.gpsimd
                )
                mul_eng.tensor_tensor(
                    out=prv, in0=e4, in1=fb, op=mybir.AluOpType.mult
                )
                nc.vector.tensor_reduce(
                    out=num_b[:, :, c, :],
                    in_=prv,
                    axis=mybir.AxisListType.X,
                    op=mybir.AluOpType.add,
                )
            ds = dpool.tile([128, 3, 64], F32, tag="dsum")
            nc.vector.tensor_reduce(
                out=ds[:], in_=e4, axis=mybir.AxisListType.X,
                op=mybir.AluOpType.add,
            )
            di = dpool.tile([128, 3, 64], F32, tag="dinv")
            nc.vector.reciprocal_approx_fast(di[:], ds[:])

            up_b = upool.tile([128, 3, 2, 64], F32, tag="up")
            nc.vector.tensor_tensor(
                out=up_b[:],
                in0=num_b[:],
                in1=di[:, :, None, :].to_broadcast((128, 3, 2, 64)),
                op=mybir.AluOpType.mult,
            )

            if os.environ.get("CONVUP_LINEAR_STORE"):
                # debug knob: contiguous (wrong-layout) store to measure the
                # cost of the pixel-shuffle scatter
                of = out.rearrange("c h w -> (c h w)")
                nc.sync.dma_start(
                    of[hb * 49152 : (hb + 1) * 49152].rearrange(
                        "(p f) -> p f", f=384
                    ),
                    up_b[:].rearrange("p t c q -> p (t c q)"),
                )
                return
            for t in range(3):
                j = 3 * hb + t
                for i0, n, h, w0 in _chunk_segments(j):
                    for c in range(2):
                        nc.sync.dma_start(
                            out_v[c, h, :, w0 : w0 + n, :].rearrange(
                                "p w q -> w p q"
                            ),
                            up_b[i0 : i0 + n, t, c, :].rearrange(
                                "w (p q) -> w p q", q=UP
                            ),
                        )

        # Software pipeline: emit band hb's conv1 before band hb-1's tail so
        # the (in-order) PE stream never stalls waiting on the ACT/DVE tail.
        # reps > 1 repeats the whole computation (timing tool only).
        for _ in range(reps):
            xb_prev = None
            for hb in range(NBAND):
                xb = conv1_band(hb)
                if xb_prev is not None:
                    tail_band(hb - 1, xb_prev)
                xb_prev = xb
            tail_band(NBAND - 1, xb_prev)
    nc.compile()
    return nc


_NC_CACHE = {}


def _get_nc(with_b2: bool, with_b1: bool = True):
    key = (with_b2, with_b1)
    if key not in _NC_CACHE:
        _NC_CACHE[key] = _build(with_b2, with_b1=with_b1)
    return _NC_CACHE[key]


def _prep_core(feat_b, flow_b):
    """Per-core input arrays from one batch element."""
    # feat: [256, 96, 96] -> padded grid [128p, 2kc, 100*98] with one extra
    # leading zero row (grid row r lives at flat (r+1)*98).
    featp = np.zeros((2, 128, 100 * GW), np.float32)
    grid = np.zeros((C, GW, GW), np.float32)
    grid[:, 1 : H + 1, 1 : W + 1] = feat_b
    featp[:, :, GW : GW + GW * GW] = grid.reshape(2, 128, GW * GW)
    featp = _mm_np(featp.transpose(1, 0, 2))

    # flow unfold (zero-padded 3x3 shifts), pre-scaled by UP=8:
    # fdat[i, j, c*9+k] = 8 * flow_pad[c, h+kh, w+kw] for pixel j*128+i
    fpad = np.zeros((2, H + 2, W + 2), np.float32)
    fpad[:, 1:-1, 1:-1] = flow_b
    shifts = np.stack(
        [fpad[:, i : i + H, j : j + W] for i in range(3) for j in range(3)],
        axis=1,
    )  # [2, 9, 96, 96]
    fdat = (8.0 * shifts).transpose(2, 3, 0, 1).reshape(PIX, 18)
    fdat = np.ascontiguousarray(fdat.reshape(NCHUNK, 128, 18).transpose(1, 0, 2))
    return featp, fdat


def _weight_args(inputs):
    """Re-laid-out weight arrays shared across cores."""
    w1 = np.asarray(inputs["w1"], np.float32)
    b1 = np.asarray(inputs["b1"], np.float32)
    w2 = np.asarray(inputs["w2"], np.float32)
    b2 = np.asarray(inputs["b2"], np.float32)
    # w1s[ci_in, kc, tap, mc, co_in] = w1[mc*128+co, kc*128+ci, kh, kw]
    t = w1.transpose(1, 2, 3, 0).reshape(2, 128, 3, 3, 2, 128)
    w1s = _mm_np(t.transpose(1, 0, 2, 3, 4, 5)).reshape(
        128, 2, 9, 2, 128
    )
    # w2 columns reordered to ch' = pq*9 + k (from ch = k*64 + pq)
    chp = np.arange(576)
    perm = (chp % 9) * 64 + chp // 9
    w2m = w2[:, :, 0, 0][perm]  # [576, 256]
    w2s = _mm_np(w2m.T.reshape(2, 128, 576).transpose(1, 0, 2))
    b1s = np.ascontiguousarray(b1.reshape(2, 128).T)
    args = {"w1s": w1s, "w2s": w2s, "b1s": b1s}
    if np.any(b2):
        args["b2s"] = _mm_np(b2[perm].reshape(1, 576))
    return args


def kernel(flow_lr, feat, w1, b1, w2, b2):
    flow_lr = np.asarray(flow_lr, np.float32)
    feat = np.asarray(feat, np.float32)
    inputs = {"w1": w1, "b1": b1, "w2": w2, "b2": b2}
    w_args = _weight_args(inputs)
    with_b2 = "b2s" in w_args
    with_b1 = bool(np.any(np.asarray(b1)))

    nc = _get_nc(with_b2, with_b1)
    in_maps = []
    for b in range(B):
        featp, fdat = _prep_core(feat[b], flow_lr[b])
        m = dict(w_args)
        m.update({"featp": featp, "fdat": fdat})
        in_maps.append(m)

    res = run_bass_kernel_spmd(nc, in_maps, list(range(B)))
    return np.stack([res.results[b]["out"] for b in range(B)]).astype(np.float32)



# revision 5
# speedup vs baseline: 1.1617x; 1.1617x over previous
"""ConvexUpsampler Trainium2 kernel (v2).

Per batch element b (one NeuronCore each, 8 cores):
  x    = relu(conv3x3(feat, w1) + b1)            # [256, 96, 96]
  m    = conv1x1(x, w2) + b2                     # [576, 96, 96]
  mask = softmax over k of m.reshape(9, 64, ...) # k = 3x3 tap index
  up   = sum_k mask[k,pq,hw] * unfold(flow)[c,k,hw] * 8
  out  = pixel-shuffle(up)                       # [2, 768, 768]

v2 strategy:
  * conv1 runs on the PE in fp8e4 DoubleRow mode (2 contraction rows per
    cycle) with exact weight compensation: w = fp8(w) + fp8(16*(w-fp8(w)))/16,
    the /16 carried by a second fp8 copy of feat pre-divided by 16 (exact
    exponent shift at the dominant magnitudes).  4x fewer PE cycles than
    fp32r at ~bf16 weight accuracy; feat quantization error remains ~3.6%
    rms which the softmax tolerates (measured end-to-end 1.2e-2 rel).
  * conv2 runs "swapped" (pixels on PSUM partitions) in bf16.
  * softmax tail in bf16 with channel order ch' = pq*9+k (k minor): the
    e*f products broadcast f along pq on a MIDDLE AP dim, keeping every
    operand's innermost dim packed, which enables the DVE 2x mode.
    Reductions over k are 4-level add trees (bf16 -> f32).
  * engine assignment: DVE products/num-trees/recip/final, Pool den-reduce,
    ACT exp+relu.  Knobs via CONVUP_* env vars.
"""

import os
import sys
from contextlib import ExitStack

if os.environ.get("JAX_PLATFORMS", "").strip().lower() == "cpu":
    os.environ["JAX_PLATFORMS"] = ""

sys.path.insert(0, "/opt/trn_rl_repo")

import numpy as np
import ml_dtypes

import concourse.bass as bass  # noqa: E402
import concourse.tile as tile  # noqa: E402
from concourse import bacc, mybir  # noqa: E402
from concourse.bass_utils import run_bass_kernel_spmd  # noqa: E402

F32 = mybir.dt.float32
BF16 = mybir.dt.bfloat16
FP8 = mybir.dt.float8e4
DR = mybir.MatmulPerfMode.DoubleRow

B = 8
C = 256
H = W = 96
UP = 8
PIX = H * W          # 9216
GW = 98              # padded grid width
NBAND = 24           # bands of 4 output rows
BAND_N = 4 * GW      # conv1 moving free size (392)
NCHUNK = PIX // 128  # 72 pixel chunks of 128
WS = 64.0            # weight pre-scale (folded back via ACT scale)

NP_FP8 = ml_dtypes.float8_e4m3
NP_BF16 = ml_dtypes.bfloat16

# knobs: engine assignment for flexible tail ops
DEN_ENG = os.environ.get("CONVUP_DEN", "pool")      # pool | dve
PROD1_ENG = os.environ.get("CONVUP_PROD1", "dve")   # dve | pool
FINAL_ENG = os.environ.get("CONVUP_FINAL", "pool")  # dve | pool


def _chunk_segments(j):
    """Split pixel chunk j (pixels 128j..128j+127, h-major) into runs with a
    single output row each: (i0, n, h, w0)."""
    segs = []
    i = 0
    while i < 128:
        pix = 128 * j + i
        h, w0 = divmod(pix, W)
        n = min(128 - i, W - w0)
        segs.append((i, n, h, w0))
        i += n
    return segs


def _build(with_b2: bool, reps: int = 1, with_b1: bool = True):
    nc = bacc.Bacc("TRN2", target_bir_lowering=False, debug=False)
    featp = nc.dram_tensor(
        "featp", [128, 2, 2, 100 * GW], FP8, kind="ExternalInput"
    ).ap()
    w1s = nc.dram_tensor(
        "w1s", [128, 2, 9, 2, 2, 128], FP8, kind="ExternalInput"
    ).ap()
    w2s = nc.dram_tensor("w2s", [128, 2, 576], BF16, kind="ExternalInput").ap()
    b1s = nc.dram_tensor("b1s", [128, 2], F32, kind="ExternalInput").ap()
    fdat = nc.dram_tensor("fdat", [128, NCHUNK, 2, 9], BF16, kind="ExternalInput").ap()
    b2s = None
    if with_b2:
        b2s = nc.dram_tensor("b2s", [1, 576], BF16, kind="ExternalInput").ap()
    out = nc.dram_tensor("out", [2, 768, 768], F32, kind="ExternalOutput").ap()
    # out viewed as [c, hh, p, ww, q] for the pixel-shuffle scatter store
    out_v = out.rearrange("c (hh p) (ww q) -> c hh p ww q", p=UP, q=UP)

    with tile.TileContext(nc) as tc, ExitStack() as ctx:
        cpool = ctx.enter_context(tc.tile_pool(name="const", bufs=1))
        xpool = ctx.enter_context(tc.tile_pool(name="x", bufs=3))
        epool = ctx.enter_context(tc.tile_pool(name="e", bufs=3))
        ppool = ctx.enter_context(tc.tile_pool(name="prod", bufs=2))
        tpool = ctx.enter_context(tc.tile_pool(name="tree", bufs=2))
        npool = ctx.enter_context(tc.tile_pool(name="num", bufs=2))
        dpool = ctx.enter_context(tc.tile_pool(name="d", bufs=2))
        upool = ctx.enter_context(tc.tile_pool(name="up", bufs=3))
        psum1 = ctx.enter_context(tc.tile_pool(name="ps1", bufs=2, space="PSUM"))
        psum2 = ctx.enter_context(tc.tile_pool(name="ps2", bufs=2, space="PSUM"))

        feat_sb = cpool.tile([128, 2, 2, 100 * GW], FP8, tag="feat")
        # split the big feat load so early conv1 bands start sooner
        for kc in range(2):
            for hl in range(2):
                for s0 in range(0, 100 * GW, 50 * GW):
                    nc.sync.dma_start(
                        feat_sb[:, kc, hl, s0 : s0 + 50 * GW],
                        featp[:, kc, hl, s0 : s0 + 50 * GW],
                    )
        w1_sb = cpool.tile([128, 2, 9, 2, 2, 128], FP8, tag="w1")
        nc.sync.dma_start(w1_sb[:], w1s[:])
        w2_sb = cpool.tile([128, 2, 576], BF16, tag="w2")
        nc.sync.dma_start(w2_sb[:], w2s[:])
        b1_sb = cpool.tile([128, 2], F32, tag="b1")
        nc.sync.dma_start(b1_sb[:], b1s[:])
        f_sb = cpool.tile([128, NCHUNK, 2, 9], BF16, tag="fdat")
        nc.sync.dma_start(f_sb[:], fdat[:])
        if with_b2:
            b2_sb = cpool.tile([1, 576], BF16, tag="b2")
            nc.sync.dma_start(b2_sb[:], b2s[:])
            ones_sb = cpool.tile([1, 128], BF16, tag="ones")
            nc.vector.memset(ones_sb[:], 1.0)

        def conv1_band(hb):
            r0 = 4 * hb + 1  # first output grid row of this band
            # conv1: 3x3x256->256 over 4 rows (padded width), fp8 DoubleRow.
            # Each instr contracts both kc halves (the DR pair); hl=0 is the
            # fp8 weight, hl=1 the 16x residual applied to feat/16.
            ps = psum1.tile([128, 2, 512], F32, tag="ps1")
            for mc in range(2):
                first = True
                for tap in range(9):
                    dh, dw = divmod(tap, 3)
                    s = (r0 + dh) * GW + dw - 1
                    for hl in range(2):
                        nc.tensor.matmul(
                            ps[:, mc, :BAND_N],
                            lhsT=w1_sb[:, :, tap, mc, hl, :],
                            rhs=feat_sb[:, :, hl, s : s + BAND_N],
                            start=first,
                            stop=(tap == 8 and hl == 1),
                            perf_mode=DR,
                        )
                        first = False
            # relu(x + b1) * (1/WS), compacting 98-wide padded rows to 96
            xt = xpool.tile([128, 2, 4 * W], BF16, tag="x")
            psv = ps[:, :, :BAND_N].rearrange("p m (r c) -> p m r c", c=GW)[
                :, :, :, 1 : W + 1
            ]
            if with_b1:
                for mc in range(2):
                    nc.scalar.activation(
                        out=xt[:, mc].rearrange("p (r c) -> p r c", c=W),
                        in_=psv[:, mc],
                        func=mybir.ActivationFunctionType.Relu,
                        bias=b1_sb[:, mc : mc + 1],
                        scale=1.0 / WS,
                    )
            else:
                nc.scalar.activation(
                    out=xt[:].rearrange("p m (r c) -> p m r c", c=W),
                    in_=psv,
                    func=mybir.ActivationFunctionType.Relu,
                    scale=1.0 / WS,
                )
            return xt

        def tail_band(hb, xb):
            # conv2 (swapped: pixels on partitions, bf16) + exp per chunk
            e_b = epool.tile([128, 3, 576], BF16, tag="e")
            for t in range(3):
                ps2 = psum2.tile([128, 576], F32, tag="ps2")
                for half in range(2):
                    for kc in range(2):
                        nc.tensor.matmul(
                            ps2[:, half * 288 : (half + 1) * 288],
                            lhsT=xb[:, kc, t * 128 : (t + 1) * 128],
                            rhs=w2_sb[:, kc, half * 288 : (half + 1) * 288],
                            start=(kc == 0),
                            stop=(kc == 1 and not with_b2),
                        )
                    if with_b2:
                        nc.tensor.matmul(
                            ps2[:, half * 288 : (half + 1) * 288],
                            lhsT=ones_sb[:, :],
                            rhs=b2_sb[:, half * 288 : (half + 1) * 288],
                            start=False,
                            stop=True,
                        )
                nc.scalar.activation(
                    out=e_b[:, t, :],
                    in_=ps2[:, :],
                    func=mybir.ActivationFunctionType.Exp,
                    scale=1.0 / WS,
                )
            # e viewed [128, 3, 64 pq, 9 k] (k minor, packed innermost)
            e4 = e_b[:].rearrange("p t (q k) -> p t q k", k=9)

            def tree9(v4, out_ap, tag, eng):
                """Sum v4 [128,3,64,9] over k into out_ap: bf16 tree, f32 end."""
                t1 = tpool.tile([128, 3, 64, 4], BF16, tag=f"{tag}1")
                eng.tensor_tensor(
                    out=t1[:], in0=v4[:, :, :, 0:4], in1=v4[:, :, :, 4:8],
                    op=mybir.AluOpType.add,
                )
                t2 = tpool.tile([128, 3, 64, 2], BF16, tag=f"{tag}2")
                eng.tensor_tensor(
                    out=t2[:], in0=t1[:, :, :, 0:2], in1=t1[:, :, :, 2:4],
                    op=mybir.AluOpType.add,
                )
                t3 = tpool.tile([128, 3, 64], F32, tag=f"{tag}3")
                eng.tensor_tensor(
                    out=t3[:], in0=t2[:, :, :, 0], in1=t2[:, :, :, 1],
                    op=mybir.AluOpType.add,
                )
                eng.tensor_tensor(
                    out=out_ap, in0=t3[:], in1=v4[:, :, :, 8],
                    op=mybir.AluOpType.add,
                )

            # numerators: products (DVE 2x: all innermost dims packed) + tree
            num_b = npool.tile([128, 3, 2, 64], F32, tag="num")
            for c in range(2):
                pr = ppool.tile([128, 3, 576], BF16, tag=f"prod{c}")
                prv = pr[:].rearrange("p t (q k) -> p t q k", k=9)
                fb = f_sb[:, 3 * hb : 3 * hb + 3, c, None, :]
                fb = fb.to_broadcast((128, 3, 64, 9))
                eng = nc.gpsimd if (c == 1 and PROD1_ENG == "pool") else nc.vector
                eng.tensor_tensor(out=prv, in0=e4, in1=fb, op=mybir.AluOpType.mult)
                tree9(prv, num_b[:, :, c, :], f"n{c}", eng=nc.vector)

            # denominator tree (knob: pool offloads it from the vector engine)
            ds = dpool.tile([128, 3, 64], F32, tag="dsum")
            tree9(e4, ds[:], "dt",
                  eng=nc.gpsimd if DEN_ENG == "pool" else nc.vector)
            di = dpool.tile([128, 3, 64], F32, tag="dinv")
            nc.vector.reciprocal_approx_fast(di[:], ds[:])

            up_b = upool.tile([128, 3, 2, 64], F32, tag="up")
            feng = nc.gpsimd if FINAL_ENG == "pool" else nc.vector
            feng.tensor_tensor(
                out=up_b[:],
                in0=num_b[:],
                in1=di[:, :, None, :].to_broadcast((128, 3, 2, 64)),
                op=mybir.AluOpType.mult,
            )

            for t in range(3):
                j = 3 * hb + t
                for i0, n, h, w0 in _chunk_segments(j):
                    for c in range(2):
                        nc.sync.dma_start(
                            out_v[c, h, :, w0 : w0 + n, :].rearrange(
                                "p w q -> w p q"
                            ),
                            up_b[i0 : i0 + n, t, c, :].rearrange(
                                "w (p q) -> w p q", q=UP
                            ),
                        )

        # Software pipeline: emit band hb's conv1 before band hb-1's tail so
        # the (in-order) PE stream never stalls waiting on the ACT/DVE tail.
        for _ in range(reps):
            xb_prev = None
            for hb in range(NBAND):
                xb = conv1_band(hb)
                if xb_prev is not None:
                    tail_band(hb - 1, xb_prev)
                xb_prev = xb
            tail_band(NBAND - 1, xb_prev)
    nc.compile()
    return nc


_NC_CACHE = {}


def _get_nc(with_b2: bool, with_b1: bool = True):
    key = (with_b2, with_b1)
    if key not in _NC_CACHE:
        _NC_CACHE[key] = _build(with_b2, with_b1=with_b1)
    return _NC_CACHE[key]


def _prep_core(feat_b, flow_b):
    """Per-core input arrays from one batch element."""
    # feat: [256, 96, 96] -> padded fp8 grid [128p, 2kc, 2hl, 100*GW] with one
    # extra leading zero row (grid row r lives at flat (r+1)*98).
    # hl=0: fp8(feat); hl=1: fp8(feat/16) for the weight-residual term.
    featp = np.zeros((2, 2, 128, 100 * GW), NP_FP8)
    grid = np.zeros((C, GW, GW), np.float32)
    grid[:, 1 : H + 1, 1 : W + 1] = feat_b
    g2 = grid.reshape(2, 128, GW * GW)
    featp[:, 0, :, GW : GW + GW * GW] = g2.astype(NP_FP8)
    featp[:, 1, :, GW : GW + GW * GW] = (g2 / 16.0).astype(NP_FP8)
    featp = np.ascontiguousarray(featp.transpose(2, 0, 1, 3))

    # flow unfold (zero-padded 3x3 shifts), pre-scaled by UP=8:
    # fdat[i, j, c, k] = 8 * flow_pad[c, h+kh, w+kw] for pixel j*128+i
    fpad = np.zeros((2, H + 2, W + 2), np.float32)
    fpad[:, 1:-1, 1:-1] = flow_b
    shifts = np.stack(
        [fpad[:, i : i + H, j : j + W] for i in range(3) for j in range(3)],
        axis=1,
    )  # [2, 9, 96, 96]
    fdat = (8.0 * shifts).transpose(2, 3, 0, 1).reshape(NCHUNK, 128, 2, 9)
    fdat = np.ascontiguousarray(fdat.transpose(1, 0, 2, 3)).astype(NP_BF16)
    return featp, fdat


def _weight_args(inputs):
    """Re-laid-out weight arrays shared across cores."""
    w1 = np.asarray(inputs["w1"], np.float32)
    b1 = np.asarray(inputs["b1"], np.float32)
    w2 = np.asarray(inputs["w2"], np.float32)
    b2 = np.asarray(inputs["b2"], np.float32)
    # w1s[ci_in, kc, tap, mc, hl, co] = scaled w1[mc*128+co, kc*128+ci, kh, kw]
    t = (w1 * WS).transpose(1, 2, 3, 0).reshape(2, 128, 3, 3, 2, 128)
    t = t.transpose(1, 0, 2, 3, 4, 5).reshape(128, 2, 9, 2, 128)
    hi = t.astype(NP_FP8)
    lo = ((t - hi.astype(np.float32)) * 16.0).astype(NP_FP8)
    w1s = np.stack([hi, lo], axis=4)  # [128, 2, 9, 2, 2hl, 128]
    w1s = np.ascontiguousarray(w1s)
    # w2 columns reordered to ch' = pq*9 + k (k minor, from ch = k*64 + pq)
    chp = np.arange(576)
    perm = (chp % 9) * 64 + chp // 9
    w2m = (w2[:, :, 0, 0] * WS)[perm]  # [576, 256]
    w2s = np.ascontiguousarray(
        w2m.T.reshape(2, 128, 576).transpose(1, 0, 2)
    ).astype(NP_BF16)
    b1s = np.ascontiguousarray((b1 * 1.0).reshape(2, 128).T)
    args = {"w1s": w1s, "w2s": w2s, "b1s": b1s}
    if np.any(b2):
        args["b2s"] = (b2[perm] * WS).reshape(1, 576).astype(NP_BF16)
    return args


def kernel(flow_lr, feat, w1, b1, w2, b2):
    flow_lr = np.asarray(flow_lr, np.float32)
    feat = np.asarray(feat, np.float32)
    inputs = {"w1": w1, "b1": b1, "w2": w2, "b2": b2}
    w_args = _weight_args(inputs)
    with_b2 = "b2s" in w_args
    with_b1 = bool(np.any(np.asarray(b1)))

    nc = _get_nc(with_b2, with_b1)
    in_maps = []
    for b in range(B):
        featp, fdat = _prep_core(feat[b], flow_lr[b])
        m = dict(w_args)
        m.update({"featp": featp, "fdat": fdat})
        in_maps.append(m)

    res = run_bass_kernel_spmd(nc, in_maps, list(range(B)))
    return np.stack([res.results[b]["out"] for b in range(B)]).astype(np.float32)


# revision 10
# speedup vs baseline: 1.7932x; 1.5436x over previous
"""ConvexUpsampler Trainium2 kernel (v2).

Per batch element b (one NeuronCore each, 8 cores):
  x    = relu(conv3x3(feat, w1) + b1)            # [256, 96, 96]
  m    = conv1x1(x, w2) + b2                     # [576, 96, 96]
  mask = softmax over k of m.reshape(9, 64, ...) # k = 3x3 tap index
  up   = sum_k mask[k,pq,hw] * unfold(flow)[c,k,hw] * 8
  out  = pixel-shuffle(up)                       # [2, 768, 768]

v2 strategy:
  * conv1 runs on the PE in fp8e4 DoubleRow mode (2 contraction rows per
    cycle) with exact weight compensation: w = fp8(w) + fp8(16*(w-fp8(w)))/16,
    the /16 carried by a second fp8 copy of feat pre-divided by 16 (exact
    exponent shift at the dominant magnitudes).  4x fewer PE cycles than
    fp32r at ~bf16 weight accuracy; feat quantization error remains ~3.6%
    rms which the softmax tolerates (measured end-to-end 1.2e-2 rel).
  * conv2 runs "swapped" (pixels on PSUM partitions) in bf16.
  * softmax tail in bf16 with channel order ch' = pq*9+k (k minor): the
    e*f products broadcast f along pq on a MIDDLE AP dim, keeping every
    operand's innermost dim packed, which enables the DVE 2x mode.
    Reductions over k are 4-level add trees (bf16 -> f32).
  * engine assignment: DVE products/num-trees/recip/final, Pool den-reduce,
    ACT exp+relu.  Knobs via CONVUP_* env vars.
"""

import os
import sys
from contextlib import ExitStack

if os.environ.get("JAX_PLATFORMS", "").strip().lower() == "cpu":
    os.environ["JAX_PLATFORMS"] = ""

sys.path.insert(0, "/opt/trn_rl_repo")

import numpy as np
import ml_dtypes

import concourse.bass as bass  # noqa: E402
import concourse.tile as tile  # noqa: E402
from concourse import bacc, mybir  # noqa: E402
from concourse.bass_utils import run_bass_kernel_spmd  # noqa: E402

F32 = mybir.dt.float32
BF16 = mybir.dt.bfloat16
FP8 = mybir.dt.float8e4
DR = mybir.MatmulPerfMode.DoubleRow

B = 8
C = 256
H = W = 96
UP = 8
PIX = H * W          # 9216
GW = 98              # padded grid width
NBAND = 24           # bands of 4 output rows
BAND_N = 4 * GW      # conv1 moving free size (392)
NCHUNK = PIX // 128  # 72 pixel chunks of 128
WS = 64.0            # weight pre-scale (folded back via ACT scale)

NP_FP8 = ml_dtypes.float8_e4m3
NP_BF16 = ml_dtypes.bfloat16

# knobs: engine assignment for flexible tail ops
DEN_ENG = os.environ.get("CONVUP_DEN", "pool")      # pool | dve
PROD1_ENG = os.environ.get("CONVUP_PROD1", "dve")   # dve | pool
FINAL_ENG = os.environ.get("CONVUP_FINAL", "pool")  # dve | pool


def _chunk_segments(j):
    """Split pixel chunk j (pixels 128j..128j+127, h-major) into runs with a
    single output row each: (i0, n, h, w0)."""
    segs = []
    i = 0
    while i < 128:
        pix = 128 * j + i
        h, w0 = divmod(pix, W)
        n = min(128 - i, W - w0)
        segs.append((i, n, h, w0))
        i += n
    return segs


def _build(with_b2: bool, reps: int = 1, with_b1: bool = True):
    nc = bacc.Bacc("TRN2", target_bir_lowering=False, debug=False)
    featp = nc.dram_tensor(
        "featp", [128, 2, 2, 100 * GW], FP8, kind="ExternalInput"
    ).ap()
    w1s = nc.dram_tensor(
        "w1s", [128, 2, 9, 2, 2, 128], FP8, kind="ExternalInput"
    ).ap()
    w2s = nc.dram_tensor("w2s", [128, 2, 576], BF16, kind="ExternalInput").ap()
    b1s = nc.dram_tensor("b1s", [128, 2], F32, kind="ExternalInput").ap()
    fdat = nc.dram_tensor("fdat", [128, NCHUNK, 2, 9], BF16, kind="ExternalInput").ap()
    b2s = None
    if with_b2:
        b2s = nc.dram_tensor("b2s", [1, 576], BF16, kind="ExternalInput").ap()
    # chunk-linear staging layout; the host applies the pixel shuffle.
    # One contiguous DMA per band instead of ~12 scatter descriptor sets.
    out = nc.dram_tensor(
        "out", [NBAND, 128, 3 * 2 * 64], F32, kind="ExternalOutput"
    ).ap()

    with tile.TileContext(nc) as tc, ExitStack() as ctx:
        cpool = ctx.enter_context(tc.tile_pool(name="const", bufs=1))
        xpool = ctx.enter_context(tc.tile_pool(name="x", bufs=3))
        epool = ctx.enter_context(tc.tile_pool(name="e", bufs=3))
        ppool = ctx.enter_context(tc.tile_pool(name="prod", bufs=2))
        tpool = ctx.enter_context(tc.tile_pool(name="tree", bufs=2))
        npool = ctx.enter_context(tc.tile_pool(name="num", bufs=2))
        dpool = ctx.enter_context(tc.tile_pool(name="d", bufs=2))
        upool = ctx.enter_context(tc.tile_pool(name="up", bufs=3))
        psum1 = ctx.enter_context(tc.tile_pool(name="ps1", bufs=2, space="PSUM"))
        psum2 = ctx.enter_context(tc.tile_pool(name="ps2", bufs=2, space="PSUM"))

        feat_sb = cpool.tile([128, 2, 2, 100 * GW], FP8, tag="feat")
        # split the big feat load so early conv1 bands start sooner
        for kc in range(2):
            for hl in range(2):
                for s0 in range(0, 100 * GW, 50 * GW):
                    nc.sync.dma_start(
                        feat_sb[:, kc, hl, s0 : s0 + 50 * GW],
                        featp[:, kc, hl, s0 : s0 + 50 * GW],
                    )
        w1_sb = cpool.tile([128, 2, 9, 2, 2, 128], FP8, tag="w1")
        nc.sync.dma_start(w1_sb[:], w1s[:])
        w2_sb = cpool.tile([128, 2, 576], BF16, tag="w2")
        nc.sync.dma_start(w2_sb[:], w2s[:])
        b1_sb = cpool.tile([128, 2], F32, tag="b1")
        nc.sync.dma_start(b1_sb[:], b1s[:])
        f_sb = cpool.tile([128, NCHUNK, 2, 9], BF16, tag="fdat")
        nc.sync.dma_start(f_sb[:], fdat[:])
        if with_b2:
            b2_sb = cpool.tile([1, 576], BF16, tag="b2")
            nc.sync.dma_start(b2_sb[:], b2s[:])
            ones_sb = cpool.tile([1, 128], BF16, tag="ones")
            nc.vector.memset(ones_sb[:], 1.0)

        def conv1_band(hb):
            r0 = 4 * hb + 1  # first output grid row of this band
            # conv1: 3x3x256->256 over 4 rows (padded width), fp8 DoubleRow.
            # Each instr contracts both kc halves (the DR pair); hl=0 is the
            # fp8 weight, hl=1 the 16x residual applied to feat/16.
            ps = psum1.tile([128, 2, 512], F32, tag="ps1")
            for mc in range(2):
                first = True
                for tap in range(9):
                    dh, dw = divmod(tap, 3)
                    s = (r0 + dh) * GW + dw - 1
                    for hl in range(2):
                        nc.tensor.matmul(
                            ps[:, mc, :BAND_N],
                            lhsT=w1_sb[:, :, tap, mc, hl, :],
                            rhs=feat_sb[:, :, hl, s : s + BAND_N],
                            start=first,
                            stop=(tap == 8 and hl == 1),
                            perf_mode=DR,
                        )
                        first = False
            # relu(x + b1) * (1/WS), compacting 98-wide padded rows to 96
            xt = xpool.tile([128, 2, 4 * W], BF16, tag="x")
            psv = ps[:, :, :BAND_N].rearrange("p m (r c) -> p m r c", c=GW)[
                :, :, :, 1 : W + 1
            ]
            if with_b1:
                for mc in range(2):
                    nc.scalar.activation(
                        out=xt[:, mc].rearrange("p (r c) -> p r c", c=W),
                        in_=psv[:, mc],
                        func=mybir.ActivationFunctionType.Relu,
                        bias=b1_sb[:, mc : mc + 1],
                        scale=1.0 / WS,
                    )
            else:
                nc.scalar.activation(
                    out=xt[:].rearrange("p m (r c) -> p m r c", c=W),
                    in_=psv,
                    func=mybir.ActivationFunctionType.Relu,
                    scale=1.0 / WS,
                )
            return xt

        def tail_band(hb, xb):
            # conv2 (swapped: pixels on partitions, bf16) + exp per chunk
            e_b = epool.tile([128, 3, 576], BF16, tag="e")
            for t in range(3):
                ps2 = psum2.tile([128, 576], F32, tag="ps2")
                for half in range(2):
                    for kc in range(2):
                        nc.tensor.matmul(
                            ps2[:, half * 288 : (half + 1) * 288],
                            lhsT=xb[:, kc, t * 128 : (t + 1) * 128],
                            rhs=w2_sb[:, kc, half * 288 : (half + 1) * 288],
                            start=(kc == 0),
                            stop=(kc == 1 and not with_b2),
                        )
                    if with_b2:
                        nc.tensor.matmul(
                            ps2[:, half * 288 : (half + 1) * 288],
                            lhsT=ones_sb[:, :],
                            rhs=b2_sb[:, half * 288 : (half + 1) * 288],
                            start=False,
                            stop=True,
                        )
                nc.scalar.activation(
                    out=e_b[:, t, :],
                    in_=ps2[:, :],
                    func=mybir.ActivationFunctionType.Exp,
                    scale=1.0 / WS,
                )
            # e viewed [128, 3, 64 pq, 9 k] (k minor, packed innermost)
            e4 = e_b[:].rearrange("p t (q k) -> p t q k", k=9)

            def tree9(v4, out_ap, tag, eng):
                """Sum v4 [128,3,64,9] over k into out_ap: bf16 tree, f32 end."""
                t1 = tpool.tile([128, 3, 64, 4], BF16, tag=f"{tag}1")
                eng.tensor_tensor(
                    out=t1[:], in0=v4[:, :, :, 0:4], in1=v4[:, :, :, 4:8],
                    op=mybir.AluOpType.add,
                )
                t2 = tpool.tile([128, 3, 64, 2], BF16, tag=f"{tag}2")
                eng.tensor_tensor(
                    out=t2[:], in0=t1[:, :, :, 0:2], in1=t1[:, :, :, 2:4],
                    op=mybir.AluOpType.add,
                )
                t3 = tpool.tile([128, 3, 64], F32, tag=f"{tag}3")
                eng.tensor_tensor(
                    out=t3[:], in0=t2[:, :, :, 0], in1=t2[:, :, :, 1],
                    op=mybir.AluOpType.add,
                )
                eng.tensor_tensor(
                    out=out_ap, in0=t3[:], in1=v4[:, :, :, 8],
                    op=mybir.AluOpType.add,
                )

            # numerators: products (DVE 2x: all innermost dims packed) + tree
            num_b = npool.tile([128, 3, 2, 64], F32, tag="num")
            for c in range(2):
                pr = ppool.tile([128, 3, 576], BF16, tag=f"prod{c}")
                prv = pr[:].rearrange("p t (q k) -> p t q k", k=9)
                fb = f_sb[:, 3 * hb : 3 * hb + 3, c, None, :]
                fb = fb.to_broadcast((128, 3, 64, 9))
                eng = nc.gpsimd if (c == 1 and PROD1_ENG == "pool") else nc.vector
                eng.tensor_tensor(out=prv, in0=e4, in1=fb, op=mybir.AluOpType.mult)
                tree9(prv, num_b[:, :, c, :], f"n{c}", eng=nc.vector)

            # denominator tree (knob: pool offloads it from the vector engine)
            ds = dpool.tile([128, 3, 64], F32, tag="dsum")
            tree9(e4, ds[:], "dt",
                  eng=nc.gpsimd if DEN_ENG == "pool" else nc.vector)
            di = dpool.tile([128, 3, 64], F32, tag="dinv")
            nc.vector.reciprocal_approx_fast(di[:], ds[:])

            up_b = upool.tile([128, 3, 2, 64], F32, tag="up")
            feng = nc.gpsimd if FINAL_ENG == "pool" else nc.vector
            feng.tensor_tensor(
                out=up_b[:],
                in0=num_b[:],
                in1=di[:, :, None, :].to_broadcast((128, 3, 2, 64)),
                op=mybir.AluOpType.mult,
            )

            # store: one contiguous DMA per band (host un-shuffles later)
            nc.sync.dma_start(
                out[hb, :, :],
                up_b[:].rearrange("w t c pq -> w (t c pq)"),
            )

        # Software pipeline: emit band hb's conv1 before band hb-1's tail so
        # the (in-order) PE stream never stalls waiting on the ACT/DVE tail.
        for _ in range(reps):
            xb_prev = None
            for hb in range(NBAND):
                xb = conv1_band(hb)
                if xb_prev is not None:
                    tail_band(hb - 1, xb_prev)
                xb_prev = xb
            tail_band(NBAND - 1, xb_prev)
    nc.compile()
    return nc


_NC_CACHE = {}


def _get_nc(with_b2: bool, with_b1: bool = True):
    key = (with_b2, with_b1)
    if key not in _NC_CACHE:
        _NC_CACHE[key] = _build(with_b2, with_b1=with_b1)
    return _NC_CACHE[key]


def _prep_core(feat_b, flow_b):
    """Per-core input arrays from one batch element."""
    # feat: [256, 96, 96] -> padded fp8 grid [128p, 2kc, 2hl, 100*GW] with one
    # extra leading zero row (grid row r lives at flat (r+1)*98).
    # hl=0: fp8(feat); hl=1: fp8(feat/16) for the weight-residual term.
    featp = np.zeros((2, 2, 128, 100 * GW), NP_FP8)
    grid = np.zeros((C, GW, GW), np.float32)
    grid[:, 1 : H + 1, 1 : W + 1] = feat_b
    g2 = grid.reshape(2, 128, GW * GW)
    featp[:, 0, :, GW : GW + GW * GW] = g2.astype(NP_FP8)
    featp[:, 1, :, GW : GW + GW * GW] = (g2 / 16.0).astype(NP_FP8)
    featp = np.ascontiguousarray(featp.transpose(2, 0, 1, 3))

    # flow unfold (zero-padded 3x3 shifts), pre-scaled by UP=8:
    # fdat[i, j, c, k] = 8 * flow_pad[c, h+kh, w+kw] for pixel j*128+i
    fpad = np.zeros((2, H + 2, W + 2), np.float32)
    fpad[:, 1:-1, 1:-1] = flow_b
    shifts = np.stack(
        [fpad[:, i : i + H, j : j + W] for i in range(3) for j in range(3)],
        axis=1,
    )  # [2, 9, 96, 96]
    fdat = (8.0 * shifts).transpose(2, 3, 0, 1).reshape(NCHUNK, 128, 2, 9)
    fdat = np.ascontiguousarray(fdat.transpose(1, 0, 2, 3)).astype(NP_BF16)
    return featp, fdat


def _weight_args(inputs):
    """Re-laid-out weight arrays shared across cores."""
    w1 = np.asarray(inputs["w1"], np.float32)
    b1 = np.asarray(inputs["b1"], np.float32)
    w2 = np.asarray(inputs["w2"], np.float32)
    b2 = np.asarray(inputs["b2"], np.float32)
    # w1s[ci_in, kc, tap, mc, hl, co] = scaled w1[mc*128+co, kc*128+ci, kh, kw]
    t = (w1 * WS).transpose(1, 2, 3, 0).reshape(2, 128, 3, 3, 2, 128)
    t = t.transpose(1, 0, 2, 3, 4, 5).reshape(128, 2, 9, 2, 128)
    hi = t.astype(NP_FP8)
    lo = ((t - hi.astype(np.float32)) * 16.0).astype(NP_FP8)
    w1s = np.stack([hi, lo], axis=4)  # [128, 2, 9, 2, 2hl, 128]
    w1s = np.ascontiguousarray(w1s)
    # w2 columns reordered to ch' = pq*9 + k (k minor, from ch = k*64 + pq)
    chp = np.arange(576)
    perm = (chp % 9) * 64 + chp // 9
    w2m = (w2[:, :, 0, 0] * WS)[perm]  # [576, 256]
    w2s = np.ascontiguousarray(
        w2m.T.reshape(2, 128, 576).transpose(1, 0, 2)
    ).astype(NP_BF16)
    b1s = np.ascontiguousarray((b1 * 1.0).reshape(2, 128).T)
    args = {"w1s": w1s, "w2s": w2s, "b1s": b1s}
    if np.any(b2):
        args["b2s"] = (b2[perm] * WS).reshape(1, 576).astype(NP_BF16)
    return args


def kernel(flow_lr, feat, w1, b1, w2, b2):
    flow_lr = np.asarray(flow_lr, np.float32)
    feat = np.asarray(feat, np.float32)
    inputs = {"w1": w1, "b1": b1, "w2": w2, "b2": b2}
    w_args = _weight_args(inputs)
    with_b2 = "b2s" in w_args
    with_b1 = bool(np.any(np.asarray(b1)))

    nc = _get_nc(with_b2, with_b1)
    in_maps = []
    for b in range(B):
        featp, fdat = _prep_core(feat[b], flow_lr[b])
        m = dict(w_args)
        m.update({"featp": featp, "fdat": fdat})
        in_maps.append(m)

    res = run_bass_kernel_spmd(nc, in_maps, list(range(B)))
    outs = []
    for b in range(B):
        stg = np.asarray(res.results[b]["out"], np.float32)
        # staging [band, i, (t, c, p*8+q)] -> pixel-major [9216, 2, 8, 8]
        v = stg.reshape(NBAND, 128, 3, 2, 8, 8).transpose(0, 2, 1, 3, 4, 5)
        v = v.reshape(H, W, 2, UP, UP)  # flat pixel = h*96+w
        outs.append(
            v.transpose(2, 0, 3, 1, 4).reshape(2, H * UP, W * UP)
        )
    return np.stack(outs).astype(np.float32)


# revision 14
# speedup vs baseline: 1.8055x; 1.0069x over previous
"""ConvexUpsampler Trainium2 kernel (v2).

Per batch element b (one NeuronCore each, 8 cores):
  x    = relu(conv3x3(feat, w1) + b1)            # [256, 96, 96]
  m    = conv1x1(x, w2) + b2                     # [576, 96, 96]
  mask = softmax over k of m.reshape(9, 64, ...) # k = 3x3 tap index
  up   = sum_k mask[k,pq,hw] * unfold(flow)[c,k,hw] * 8
  out  = pixel-shuffle(up)                       # [2, 768, 768]

v2 strategy:
  * conv1 runs on the PE in fp8e4 DoubleRow mode (2 contraction rows per
    cycle) with exact weight compensation: w = fp8(w) + fp8(16*(w-fp8(w)))/16,
    the /16 carried by a second fp8 copy of feat pre-divided by 16 (exact
    exponent shift at the dominant magnitudes).  4x fewer PE cycles than
    fp32r at ~bf16 weight accuracy; feat quantization error remains ~3.6%
    rms which the softmax tolerates (measured end-to-end 1.2e-2 rel).
  * conv2 runs "swapped" (pixels on PSUM partitions) in bf16.
  * softmax tail in bf16 with channel order ch' = pq*9+k (k minor): the
    e*f products broadcast f along pq on a MIDDLE AP dim, keeping every
    operand's innermost dim packed, which enables the DVE 2x mode.
    Reductions over k are 4-level add trees (bf16 -> f32).
  * engine assignment: DVE products/num-trees/recip/final, Pool den-reduce,
    ACT exp+relu.  Knobs via CONVUP_* env vars.
"""

import os
import sys
from contextlib import ExitStack

if os.environ.get("JAX_PLATFORMS", "").strip().lower() == "cpu":
    os.environ["JAX_PLATFORMS"] = ""

sys.path.insert(0, "/opt/trn_rl_repo")

import numpy as np
import ml_dtypes

import concourse.bass as bass  # noqa: E402
import concourse.tile as tile  # noqa: E402
from concourse import bacc, mybir  # noqa: E402
from concourse.bass_utils import run_bass_kernel_spmd  # noqa: E402

F32 = mybir.dt.float32
BF16 = mybir.dt.bfloat16
FP8 = mybir.dt.float8e4
DR = mybir.MatmulPerfMode.DoubleRow

B = 8
C = 256
H = W = 96
UP = 8
PIX = H * W          # 9216
GW = 98              # padded grid width
NBAND = 24           # bands of 4 output rows
BAND_N = 4 * GW      # conv1 moving free size (392)
NCHUNK = PIX // 128  # 72 pixel chunks of 128
WS = 64.0            # weight pre-scale (folded back via ACT scale)

NP_FP8 = ml_dtypes.float8_e4m3
NP_BF16 = ml_dtypes.bfloat16

# knobs: engine assignment for flexible tail ops
DEN_ENG = os.environ.get("CONVUP_DEN", "pool")      # pool | dve
PROD1_ENG = os.environ.get("CONVUP_PROD1", "dve")   # dve | pool
FINAL_ENG = os.environ.get("CONVUP_FINAL", "pool")  # dve | pool


def _chunk_segments(j):
    """Split pixel chunk j (pixels 128j..128j+127, h-major) into runs with a
    single output row each: (i0, n, h, w0)."""
    segs = []
    i = 0
    while i < 128:
        pix = 128 * j + i
        h, w0 = divmod(pix, W)
        n = min(128 - i, W - w0)
        segs.append((i, n, h, w0))
        i += n
    return segs


def _build(with_b2: bool, reps: int = 1, with_b1: bool = True):
    nc = bacc.Bacc("TRN2", target_bir_lowering=False, debug=False)
    featp = nc.dram_tensor(
        "featp", [128, 2, 2, 100 * GW], FP8, kind="ExternalInput"
    ).ap()
    w1s = nc.dram_tensor(
        "w1s", [128, 2, 9, 2, 2, 128], FP8, kind="ExternalInput"
    ).ap()
    w2s = nc.dram_tensor("w2s", [128, 2, 576], BF16, kind="ExternalInput").ap()
    b1s = nc.dram_tensor("b1s", [128, 2], F32, kind="ExternalInput").ap()
    fdat = nc.dram_tensor("fdat", [128, NCHUNK, 2, 9], BF16, kind="ExternalInput").ap()
    b2s = None
    if with_b2:
        b2s = nc.dram_tensor("b2s", [1, 576], BF16, kind="ExternalInput").ap()
    # chunk-linear staging layout; the host applies the pixel shuffle.
    # One contiguous DMA per band instead of ~12 scatter descriptor sets.
    out = nc.dram_tensor(
        "out", [NBAND, 128, 3 * 2 * 64], F32, kind="ExternalOutput"
    ).ap()

    with tile.TileContext(nc) as tc, ExitStack() as ctx:
        cpool = ctx.enter_context(tc.tile_pool(name="const", bufs=1))
        xpool = ctx.enter_context(tc.tile_pool(name="x", bufs=3))
        epool = ctx.enter_context(tc.tile_pool(name="e", bufs=3))
        ppool = ctx.enter_context(tc.tile_pool(name="prod", bufs=2))
        tpool = ctx.enter_context(tc.tile_pool(name="tree", bufs=2))
        npool = ctx.enter_context(tc.tile_pool(name="num", bufs=2))
        dpool = ctx.enter_context(tc.tile_pool(name="d", bufs=2))
        upool = ctx.enter_context(tc.tile_pool(name="up", bufs=3))
        psum1 = ctx.enter_context(tc.tile_pool(name="ps1", bufs=2, space="PSUM"))
        psum2 = ctx.enter_context(tc.tile_pool(name="ps2", bufs=2, space="PSUM"))

        feat_sb = cpool.tile([128, 2, 2, 100 * GW], FP8, tag="feat")
        # split the big feat load so early conv1 bands start sooner
        for kc in range(2):
            for hl in range(2):
                for s0 in range(0, 100 * GW, 50 * GW):
                    nc.sync.dma_start(
                        feat_sb[:, kc, hl, s0 : s0 + 50 * GW],
                        featp[:, kc, hl, s0 : s0 + 50 * GW],
                    )
        w1_sb = cpool.tile([128, 2, 9, 2, 2, 128], FP8, tag="w1")
        nc.sync.dma_start(w1_sb[:], w1s[:])
        w2_sb = cpool.tile([128, 2, 576], BF16, tag="w2")
        nc.sync.dma_start(w2_sb[:], w2s[:])
        b1_sb = cpool.tile([128, 2], F32, tag="b1")
        nc.sync.dma_start(b1_sb[:], b1s[:])
        f_sb = cpool.tile([128, NCHUNK, 2, 9], BF16, tag="fdat")
        nc.sync.dma_start(f_sb[:], fdat[:])
        if with_b2:
            b2_sb = cpool.tile([1, 576], BF16, tag="b2")
            nc.sync.dma_start(b2_sb[:], b2s[:])
            ones_sb = cpool.tile([1, 128], BF16, tag="ones")
            nc.vector.memset(ones_sb[:], 1.0)

        def conv1_half(hb, mc, ps, xt):
            """conv1 for one output-channel half: 3x3x256->128 over 4 rows
            (padded width), fp8 DoubleRow.  Each instr contracts both kc
            halves (the DR pair); hl=0 is the fp8 weight, hl=1 the 16x
            residual applied to feat/16.  Ends with the relu for the half."""
            r0 = 4 * hb + 1  # first output grid row of this band
            first = True
            for tap in range(9):
                dh, dw = divmod(tap, 3)
                s = (r0 + dh) * GW + dw - 1
                for hl in range(2):
                    nc.tensor.matmul(
                        ps[:, mc, :BAND_N],
                        lhsT=w1_sb[:, :, tap, mc, hl, :],
                        rhs=feat_sb[:, :, hl, s : s + BAND_N],
                        start=first,
                        stop=(tap == 8 and hl == 1),
                        perf_mode=DR,
                    )
                    first = False
            # relu(x + b1) * (1/WS), compacting 98-wide padded rows to 96
            psv = ps[:, mc, :BAND_N].rearrange("p (r c) -> p r c", c=GW)[
                :, :, 1 : W + 1
            ]
            kwargs = {"bias": b1_sb[:, mc : mc + 1]} if with_b1 else {}
            nc.scalar.activation(
                out=xt[:, mc].rearrange("p (r c) -> p r c", c=W),
                in_=psv,
                func=mybir.ActivationFunctionType.Relu,
                scale=1.0 / WS,
                **kwargs,
            )

        def conv2_band(hb, xb):
            # conv2 (swapped: pixels on partitions, bf16) + exp per chunk
            e_b = epool.tile([128, 3, 576], BF16, tag="e")
            for t in range(3):
                ps2 = psum2.tile([128, 576], F32, tag="ps2")
                for half in range(2):
                    for kc in range(2):
                        nc.tensor.matmul(
                            ps2[:, half * 288 : (half + 1) * 288],
                            lhsT=xb[:, kc, t * 128 : (t + 1) * 128],
                            rhs=w2_sb[:, kc, half * 288 : (half + 1) * 288],
                            start=(kc == 0),
                            stop=(kc == 1 and not with_b2),
                        )
                    if with_b2:
                        nc.tensor.matmul(
                            ps2[:, half * 288 : (half + 1) * 288],
                            lhsT=ones_sb[:, :],
                            rhs=b2_sb[:, half * 288 : (half + 1) * 288],
                            start=False,
                            stop=True,
                        )
                nc.scalar.activation(
                    out=e_b[:, t, :],
                    in_=ps2[:, :],
                    func=mybir.ActivationFunctionType.Exp,
                    scale=1.0 / WS,
                )
            return e_b

        def tail_band(hb, e_b):
            # e viewed [128, 3, 64 pq, 9 k] (k minor, packed innermost)
            e4 = e_b[:].rearrange("p t (q k) -> p t q k", k=9)

            def tree9(v4, out_ap, tag, eng):
                """Sum v4 [128,3,64,9] over k into out_ap: bf16 tree, f32 end."""
                t1 = tpool.tile([128, 3, 64, 4], BF16, tag=f"{tag}1")
                eng.tensor_tensor(
                    out=t1[:], in0=v4[:, :, :, 0:4], in1=v4[:, :, :, 4:8],
                    op=mybir.AluOpType.add,
                )
                t2 = tpool.tile([128, 3, 64, 2], BF16, tag=f"{tag}2")
                eng.tensor_tensor(
                    out=t2[:], in0=t1[:, :, :, 0:2], in1=t1[:, :, :, 2:4],
                    op=mybir.AluOpType.add,
                )
                t3 = tpool.tile([128, 3, 64], F32, tag=f"{tag}3")
                eng.tensor_tensor(
                    out=t3[:], in0=t2[:, :, :, 0], in1=t2[:, :, :, 1],
                    op=mybir.AluOpType.add,
                )
                eng.tensor_tensor(
                    out=out_ap, in0=t3[:], in1=v4[:, :, :, 8],
                    op=mybir.AluOpType.add,
                )

            # denominator tree first: it is the longest serial chain on the
            # Pool engine and depends only on e_b.
            ds = dpool.tile([128, 3, 64], F32, tag="dsum")
            tree9(e4, ds[:], "dt",
                  eng=nc.gpsimd if DEN_ENG == "pool" else nc.vector)

            # numerators: products (DVE 2x: all innermost dims packed) + tree
            num_b = npool.tile([128, 3, 2, 64], F32, tag="num")
            for c in range(2):
                pr = ppool.tile([128, 3, 576], BF16, tag=f"prod{c}")
                prv = pr[:].rearrange("p t (q k) -> p t q k", k=9)
                fb = f_sb[:, 3 * hb : 3 * hb + 3, c, None, :]
                fb = fb.to_broadcast((128, 3, 64, 9))
                eng = nc.gpsimd if (c == 1 and PROD1_ENG == "pool") else nc.vector
                eng.tensor_tensor(out=prv, in0=e4, in1=fb, op=mybir.AluOpType.mult)
                tree9(prv, num_b[:, :, c, :], f"n{c}", eng=nc.vector)
            di = dpool.tile([128, 3, 64], F32, tag="dinv")
            nc.vector.reciprocal_approx_fast(di[:], ds[:])

            up_b = upool.tile([128, 3, 2, 64], F32, tag="up")
            feng = nc.gpsimd if FINAL_ENG == "pool" else nc.vector
            feng.tensor_tensor(
                out=up_b[:],
                in0=num_b[:],
                in1=di[:, :, None, :].to_broadcast((128, 3, 2, 64)),
                op=mybir.AluOpType.mult,
            )

            # store: one contiguous DMA per band (host un-shuffles later)
            nc.sync.dma_start(
                out[hb, :, :],
                up_b[:].rearrange("w t c pq -> w (t c pq)"),
            )

        # Software pipeline.  Per step: half of band hb's conv1, then band
        # hb-1's conv2 (whose relu input is already available), then the
        # other conv1 half, then band hb-1's vector tail.  The PE thus
        # alternates conv1/conv2 work without waiting on ACT, and the exps
        # reach ACT as early as possible.
        for _ in range(reps):
            xb_prev = None
            for hb in range(NBAND):
                ps = psum1.tile([128, 2, 512], F32, tag="ps1")
                xt = xpool.tile([128, 2, 4 * W], BF16, tag="x")
                conv1_half(hb, 0, ps, xt)
                e_prev = conv2_band(hb - 1, xb_prev) if xb_prev is not None else None
                conv1_half(hb, 1, ps, xt)
                if e_prev is not None:
                    tail_band(hb - 1, e_prev)
                xb_prev = xt
            e_prev = conv2_band(NBAND - 1, xb_prev)
            tail_band(NBAND - 1, e_prev)
    nc.compile()
    return nc


_NC_CACHE = {}


def _get_nc(with_b2: bool, with_b1: bool = True):
    key = (with_b2, with_b1)
    if key not in _NC_CACHE:
        _NC_CACHE[key] = _build(with_b2, with_b1=with_b1)
    return _NC_CACHE[key]


def _prep_core(feat_b, flow_b):
    """Per-core input arrays from one batch element."""
    # feat: [256, 96, 96] -> padded fp8 grid [128p, 2kc, 2hl, 100*GW] with one
    # extra leading zero row (grid row r lives at flat (r+1)*98).
    # hl=0: fp8(feat); hl=1: fp8(feat/16) for the weight-residual term.
    featp = np.zeros((2, 2, 128, 100 * GW), NP_FP8)
    grid = np.zeros((C, GW, GW), np.float32)
    grid[:, 1 : H + 1, 1 : W + 1] = feat_b
    g2 = grid.reshape(2, 128, GW * GW)
    featp[:, 0, :, GW : GW + GW * GW] = g2.astype(NP_FP8)
    featp[:, 1, :, GW : GW + GW * GW] = (g2 / 16.0).astype(NP_FP8)
    featp = np.ascontiguousarray(featp.transpose(2, 0, 1, 3))

    # flow unfold (zero-padded 3x3 shifts), pre-scaled by UP=8:
    # fdat[i, j, c, k] = 8 * flow_pad[c, h+kh, w+kw] for pixel j*128+i
    fpad = np.zeros((2, H + 2, W + 2), np.float32)
    fpad[:, 1:-1, 1:-1] = flow_b
    shifts = np.stack(
        [fpad[:, i : i + H, j : j + W] for i in range(3) for j in range(3)],
        axis=1,
    )  # [2, 9, 96, 96]
    fdat = (8.0 * shifts).transpose(2, 3, 0, 1).reshape(NCHUNK, 128, 2, 9)
    fdat = np.ascontiguousarray(fdat.transpose(1, 0, 2, 3)).astype(NP_BF16)
    return featp, fdat


def _weight_args(inputs):
    """Re-laid-out weight arrays shared across cores."""
    w1 = np.asarray(inputs["w1"], np.float32)
    b1 = np.asarray(inputs["b1"], np.float32)
    w2 = np.asarray(inputs["w2"], np.float32)
    b2 = np.asarray(inputs["b2"], np.float32)
    # w1s[ci_in, kc, tap, mc, hl, co] = scaled w1[mc*128+co, kc*128+ci, kh, kw]
    t = (w1 * WS).transpose(1, 2, 3, 0).reshape(2, 128, 3, 3, 2, 128)
    t = t.transpose(1, 0, 2, 3, 4, 5).reshape(128, 2, 9, 2, 128)
    hi = t.astype(NP_FP8)
    lo = ((t - hi.astype(np.float32)) * 16.0).astype(NP_FP8)
    w1s = np.stack([hi, lo], axis=4)  # [128, 2, 9, 2, 2hl, 128]
    w1s = np.ascontiguousarray(w1s)
    # w2 columns reordered to ch' = pq*9 + k (k minor, from ch = k*64 + pq)
    chp = np.arange(576)
    perm = (chp % 9) * 64 + chp // 9
    w2m = (w2[:, :, 0, 0] * WS)[perm]  # [576, 256]
    w2s = np.ascontiguousarray(
        w2m.T.reshape(2, 128, 576).transpose(1, 0, 2)
    ).astype(NP_BF16)
    b1s = np.ascontiguousarray((b1 * 1.0).reshape(2, 128).T)
    args = {"w1s": w1s, "w2s": w2s, "b1s": b1s}
    if np.any(b2):
        args["b2s"] = (b2[perm] * WS).reshape(1, 576).astype(NP_BF16)
    return args


def kernel(flow_lr, feat, w1, b1, w2, b2):
    flow_lr = np.asarray(flow_lr, np.float32)
    feat = np.asarray(feat, np.float32)
    inputs = {"w1": w1, "b1": b1, "w2": w2, "b2": b2}
    w_args = _weight_args(inputs)
    with_b2 = "b2s" in w_args
    with_b1 = bool(np.any(np.asarray(b1)))

    nc = _get_nc(with_b2, with_b1)
    in_maps = []
    for b in range(B):
        featp, fdat = _prep_core(feat[b], flow_lr[b])
        m = dict(w_args)
        m.update({"featp": featp, "fdat": fdat})
        in_maps.append(m)

    res = run_bass_kernel_spmd(nc, in_maps, list(range(B)))
    outs = []
    for b in range(B):
        stg = np.asarray(res.results[b]["out"], np.float32)
        # staging [band, i, (t, c, p*8+q)] -> pixel-major [9216, 2, 8, 8]
        v = stg.reshape(NBAND, 128, 3, 2, 8, 8).transpose(0, 2, 1, 3, 4, 5)
        v = v.reshape(H, W, 2, UP, UP)  # flat pixel = h*96+w
        outs.append(
            v.transpose(2, 0, 3, 1, 4).reshape(2, H * UP, W * UP)
        )
    return np.stack(outs).astype(np.float32)
